# revision 5
# baseline (speedup 1.0000x reference)
"""TRN2 Bass kernel for margin-based triplet loss (nn_Criterion_28278064676994).

Sharding: triplets data-parallel across 8 NeuronCores (62500 each, padded to
65536). Per core the device computes, for each triplet chunk ([128 part x 32
slots] x 128 dims):
    d1 = a - p ; d2 = a - n            (DVE)
    s1 = d1^2 ; s2 = d2^2              (scalar engine)
    dsq = reduce_add over dims         (DVE)
then a fused tail over [128, 512]:
    d_ap = sqrt(dsq + eps), pos = relu(d_ap - b + M), neg = relu(b + M - d_an)
    s = (pos + neg) * mask, ind = s > 0, partials = [sum(s), sum(ind)]
Partials are all-reduced over partitions (gpsimd) and the 8 per-core partial
pairs are combined on host: loss = where(cnt==0, tot, tot/max(cnt,1)).
"""
import numpy as np
from contextlib import ExitStack

MARGIN = 0.2
EPS = 1e-8
NCORES = 8
D = 128
TC = 62500          # triplets per core
F = 32              # free slots per chunk per partition
CHUNK = 128 * F     # 4096 triplets per chunk
NCH = 16            # chunks per core
TPAD = NCH * CHUNK  # 65536

_CACHE = {}


def _build_runner():
    if "runner" in _CACHE:
        return _CACHE["runner"]

    import concourse.bacc as bacc
    import concourse.tile as tile
    import concourse.mybir as mybir

    nc = bacc.Bacc("TRN2", target_bir_lowering=False, debug=False,
                   num_devices=NCORES)
    f32 = mybir.dt.float32
    ag_d = nc.dram_tensor("ag", (NCH, 128, F * D), f32, kind="ExternalInput").ap()
    pg_d = nc.dram_tensor("pg", (NCH, 128, F * D), f32, kind="ExternalInput").ap()
    ng_d = nc.dram_tensor("ng", (NCH, 128, F * D), f32, kind="ExternalInput").ap()
    bv_d = nc.dram_tensor("bv", (128, NCH * F), f32, kind="ExternalInput").ap()
    mk_d = nc.dram_tensor("mk", (128, NCH * F), f32, kind="ExternalInput").ap()
    out_d = nc.dram_tensor("out", (1, 2), f32, kind="ExternalOutput").ap()

    A = mybir.AluOpType
    with tile.TileContext(nc) as tc, ExitStack() as ctx:
        sb = ctx.enter_context(tc.tile_pool(name="sb", bufs=2))
        per = ctx.enter_context(tc.tile_pool(name="per", bufs=1))

        dsq_ap = per.tile([128, NCH * F], f32)
        dsq_an = per.tile([128, NCH * F], f32)
        b_t = per.tile([128, NCH * F], f32)
        mk_t = per.tile([128, NCH * F], f32)
        nc.sync.dma_start(b_t[:], bv_d[:])
        nc.sync.dma_start(mk_t[:], mk_d[:])

        for c in range(NCH):
            ga = sb.tile([128, F, D], f32, tag="ga")
            gp = sb.tile([128, F, D], f32, tag="gp")
            gn = sb.tile([128, F, D], f32, tag="gn")
            nc.sync.dma_start(ga[:], ag_d[c])
            nc.sync.dma_start(gp[:], pg_d[c])
            nc.sync.dma_start(gn[:], ng_d[c])
            d1 = sb.tile([128, F, D], f32, tag="d1")
            d2 = sb.tile([128, F, D], f32, tag="d2")
            nc.vector.tensor_tensor(out=d1[:], in0=ga[:], in1=gp[:], op=A.subtract)
            nc.vector.tensor_tensor(out=d2[:], in0=ga[:], in1=gn[:], op=A.subtract)
            nc.scalar.activation(d1[:], d1[:], mybir.ActivationFunctionType.Square)
            nc.scalar.activation(d2[:], d2[:], mybir.ActivationFunctionType.Square)
            nc.vector.tensor_reduce(
                dsq_ap[:, c * F:(c + 1) * F], d1[:],
                axis=mybir.AxisListType.X, op=A.add)
            nc.vector.tensor_reduce(
                dsq_an[:, c * F:(c + 1) * F], d2[:],
                axis=mybir.AxisListType.X, op=A.add)

        # tail over [128, NCH*F]
        dap = per.tile([128, NCH * F], f32)
        dan = per.tile([128, NCH * F], f32)
        epsb = per.tile([128, 1], f32)
        nc.vector.memset(epsb[:], EPS)
        nc.scalar.activation(dap[:], dsq_ap[:],
                             mybir.ActivationFunctionType.Sqrt, bias=epsb[:])
        nc.scalar.activation(dan[:], dsq_an[:],
                             mybir.ActivationFunctionType.Sqrt, bias=epsb[:])
        pos = per.tile([128, NCH * F], f32)
        neg = per.tile([128, NCH * F], f32)
        # pos = (dap + M) - b ; neg = (b + M) - dan
        nc.vector.scalar_tensor_tensor(
            out=pos[:], in0=dap[:], scalar=MARGIN, in1=b_t[:],
            op0=A.add, op1=A.subtract)
        nc.vector.scalar_tensor_tensor(
            out=neg[:], in0=b_t[:], scalar=MARGIN, in1=dan[:],
            op0=A.add, op1=A.subtract)
        nc.vector.tensor_scalar_max(out=pos[:], in0=pos[:], scalar1=0.0)
        nc.vector.tensor_scalar_max(out=neg[:], in0=neg[:], scalar1=0.0)
        s_t = per.tile([128, NCH * F], f32)
        nc.vector.tensor_tensor(out=s_t[:], in0=pos[:], in1=neg[:], op=A.add)
        nc.vector.tensor_tensor(out=s_t[:], in0=s_t[:], in1=mk_t[:], op=A.mult)
        ind = per.tile([128, NCH * F], f32)
        nc.vector.tensor_scalar(out=ind[:], in0=s_t[:], scalar1=0.0,
                                scalar2=None, op0=A.is_gt)
        pr = per.tile([128, 2], f32)
        nc.vector.tensor_reduce(pr[:, 0:1], s_t[:],
                                axis=mybir.AxisListType.X, op=A.add)
        nc.vector.tensor_reduce(pr[:, 1:2], ind[:],
                                axis=mybir.AxisListType.X, op=A.add)
        import concourse.bass_isa as bass_isa
        red = per.tile([128, 2], f32)
        nc.gpsimd.partition_all_reduce(red[:], pr[:], channels=128,
                                       reduce_op=bass_isa.ReduceOp.add)
        nc.sync.dma_start(out_d[:], red[0:1, :])

    nc.compile()
    _CACHE["runner"] = (nc, _make_runner_factory(nc))
    return _CACHE["runner"]


def _make_runner_factory(nc):
    """Returns runner(in_maps) -> run_fn, mirroring bass2jax.run_bass_via_pjrt
    but with a reusable jitted callable (inputs staged on device once)."""
    import jax
    import numpy as _np
    from jax.sharding import Mesh, PartitionSpec
    from jax.experimental.shard_map import shard_map
    import concourse.mybir as mybir
    from concourse.bass2jax import (
        _bass_exec_p, install_neuronx_cc_hook, partition_id_tensor)

    install_neuronx_cc_hook()
    partition_name = nc.partition_id_tensor.name if nc.partition_id_tensor else None
    in_names, out_names, out_avals, zero_outs = [], [], [], []
    for alloc in nc.m.functions[0].allocations:
        if not isinstance(alloc, mybir.MemoryLocationSet):
            continue
        name = alloc.memorylocations[0].name
        if alloc.kind == "ExternalInput":
            if name != partition_name:
                in_names.append(name)
        elif alloc.kind == "ExternalOutput":
            out_names.append(name)
            shape = tuple(alloc.tensor_shape)
            dtype = mybir.dt.np(alloc.dtype)
            out_avals.append(jax.core.ShapedArray(shape, dtype))
            zero_outs.append(_np.zeros(shape, dtype))
    n_params, n_outs = len(in_names), len(out_avals)
    all_in = list(in_names) + list(out_names)
    if partition_name is not None:
        all_in.append(partition_name)

    def _body(*args):
        operands = list(args)
        if partition_name is not None:
            operands.append(partition_id_tensor())
        return tuple(_bass_exec_p.bind(
            *operands, out_avals=tuple(out_avals), in_names=tuple(all_in),
            out_names=tuple(out_names), lowering_input_output_aliases=(),
            sim_require_finite=True, sim_require_nnan=True, nc=nc))

    devices = jax.devices()[:NCORES]
    mesh = Mesh(_np.asarray(devices), ("core",))
    sharded = jax.jit(
        shard_map(_body, mesh=mesh,
                  in_specs=(PartitionSpec("core"),) * (n_params + n_outs),
                  out_specs=(PartitionSpec("core"),) * n_outs,
                  check_rep=False),
        keep_unused=True)
    sharding = jax.sharding.NamedSharding(mesh, PartitionSpec("core"))

    def runner(in_maps):
        concat_in = [
            _np.concatenate([_np.asarray(in_maps[c][nm]) for c in range(NCORES)],
                            axis=0)
            for nm in in_names
        ]
        dev_in = [jax.device_put(x, sharding) for x in concat_in]
        dev_zero = [
            jax.device_put(
                _np.zeros((NCORES * z.shape[0], *z.shape[1:]), z.dtype), sharding)
            for z in zero_outs
        ]

        def run_fn():
            outs = sharded(*dev_in, *dev_zero)
            jax.block_until_ready(outs)
            return [
                {nm: _np.asarray(outs[i]).reshape(NCORES, *out_avals[i].shape)[c]
                 for i, nm in enumerate(out_names)}
                for c in range(NCORES)
            ]

        return run_fn
    return runner


def _prep_core(batch, beta, labels, a, p, n, valid):
    """Build one core's input map. a/p/n/valid are padded [TPAD] arrays."""
    ash = a.reshape(NCH, 128, F)
    ag = batch[ash].reshape(NCH, 128, F * D)
    pg = batch[p.reshape(NCH, 128, F)].reshape(NCH, 128, F * D)
    ng = batch[n.reshape(NCH, 128, F)].reshape(NCH, 128, F * D)
    bv = beta[labels[ash]].transpose(1, 0, 2).reshape(128, NCH * F)
    mk = valid.reshape(NCH, 128, F).transpose(1, 0, 2).reshape(128, NCH * F)
    return {
        "ag": np.ascontiguousarray(ag),
        "pg": np.ascontiguousarray(pg),
        "ng": np.ascontiguousarray(ng),
        "bv": np.ascontiguousarray(bv.astype(np.float32)),
        "mk": np.ascontiguousarray(mk.astype(np.float32)),
    }


def kernel(batch, beta, labels, triplets):
    batch = np.asarray(batch, dtype=np.float32)
    beta = np.asarray(beta, dtype=np.float32)
    labels = np.asarray(labels).astype(np.int64)
    triplets = np.asarray(triplets).astype(np.int64)
    T = triplets.shape[0]
    assert T == NCORES * TC, (T, NCORES * TC)

    in_maps = []
    for c in range(NCORES):
        tr = triplets[c * TC:(c + 1) * TC]
        a = np.zeros(TPAD, np.int64); a[:TC] = tr[:, 0]
        p = np.zeros(TPAD, np.int64); p[:TC] = tr[:, 1]
        n = np.zeros(TPAD, np.int64); n[:TC] = tr[:, 2]
        valid = np.zeros(TPAD, np.float32); valid[:TC] = 1.0
        in_maps.append(_prep_core(batch, beta, labels, a, p, n, valid))

    nc, runner_factory = _build_runner()
    run_fn = runner_factory(in_maps)
    res = run_fn()
    tot = sum(float(r["out"][0, 0]) for r in res)
    cnt = sum(float(r["out"][0, 1]) for r in res)
    loss = tot if cnt == 0.0 else tot / max(cnt, 1.0)
    return np.float32(loss)


if __name__ == "__main__":
    # smoke test with random data
    rng = np.random.default_rng(0)
    batch = rng.standard_normal((16384, 128)).astype(np.float32)
    beta = np.full((1000,), 1.2, np.float32)
    labels = rng.integers(0, 1000, 16384)
    triplets = rng.integers(0, 16384, (500000, 3))
    out = kernel(batch=batch, beta=beta, labels=labels, triplets=triplets)
    print("loss:", out)
